# revision 9
# baseline (speedup 1.0000x reference)
"""Distributed Trainium2 kernel for nn_AttentionBlock (channel attention).

Algorithm (exact algebra, no approximation):
  attn is [C,C] contracted over N tokens; everything derives from the Gram
  matrix G = x x^T and channel sums s.  Per core: phase 1 computes local
  G (+s), an AllReduce over the 4 sequence shards, a [512,512] chain
  computes lhsT_A = (P attn Wv' diag(a))^T and delta, phase 2 computes
  out = x + A x + delta 1^T over the local tokens.

v2 speedups over the bf16 baseline:
  - x transposes for phase 1 ride the XBAR DMA-transpose (off the PE).
  - G and phase 2 run in fp8e4m3 DoubleRow (0.5 cyc/row, 2x bf16) with
    error compensation: v = hi/16 + lo/16 (hi = fp8(16 v), lo = fp8(16 v
    - hi)).  G = [hh + X + X^T]/256 with X = (hi lo^T)^T computed via one
    extra DoubleRow pass + a cheap [C,C] transpose fixup.  Phase 2 is a
    3-pass split A_hi x_hi + A_hi x_lo + A_lo x_hi at scale SA*SC.
  - AllReduce split: tiny f32 [s|diag] AR first (stats hide under the G
    AR), then the G payload in fp16 (512KB, half the bytes).
  - Weights are transposed and cast to fp16 on the host; chain matmuls
    are fp16 (1 cyc/row, fast LDWEIGHTS); A0T = M0^T PwT is computed
    directly so no second transpose round; no fp32 dummy matmuls.
  - Residual is added as bf16 x in the phase-2 epilogue (vector), bias +
    descale on the scalar engine.

Sharding: batch B=2 x sequence 4 -> 8 cores. replica groups [[0..3],[4..7]].
"""

from contextlib import ExitStack

import numpy as np

import concourse.bass as bass
import concourse.tile as tile
from concourse import bacc, mybir
from concourse.bass_utils import run_bass_kernel_spmd
from concourse.masks import make_identity

# Problem constants (hardcoded per harness contract)
B = 2
C = 512
N = 32768          # 32*32*32
NCORES = 8
SHARDS = 4         # sequence shards per batch
NS = N // SHARDS   # 8192 per-core tokens
GROUPS = 32
GSIZE = C // GROUPS  # 16
EPS = 1e-5
P = 128
CT = C // P        # 4 channel tiles
F32 = mybir.dt.float32
F16 = mybir.dt.float16
BF16 = mybir.dt.bfloat16
FP8 = mybir.dt.float8e4

SC = 16.0          # x split scale: x*16 = hi + lo
SA = 256.0         # A split scale
SCG = 1.0 / (SC * SC)   # G descale

PH1_CHUNK = 1024
PH1_ITERS = NS // PH1_CHUNK     # 8
NSUB = PH1_CHUNK // P           # 8 transposed 128-subchunks per chunk
PH2_CHUNK = 512
PH2_ITERS = NS // PH2_CHUNK     # 16

REPLICA_GROUPS = [[0, 1, 2, 3], [4, 5, 6, 7]]
SCALE = 1.0 / float(np.sqrt(C))
DR = mybir.MatmulPerfMode.DoubleRow


def build_graph():
    nc = bacc.Bacc(
        "TRN2", target_bir_lowering=False, debug=False, num_devices=NCORES
    )

    x_ext = nc.dram_tensor("x", [C, NS], F32, kind="ExternalInput")
    gn_w_ext = nc.dram_tensor("gn_w", [C], F32, kind="ExternalInput")
    gn_b_ext = nc.dram_tensor("gn_b", [C], F32, kind="ExternalInput")
    qkv_b_ext = nc.dram_tensor("qkv_b", [3 * C], F32, kind="ExternalInput")
    proj_b_ext = nc.dram_tensor("proj_b", [C], F32, kind="ExternalInput")
    wqT_ext = nc.dram_tensor("wqT", [C, C], F16, kind="ExternalInput")
    wkT_ext = nc.dram_tensor("wkT", [C, C], F16, kind="ExternalInput")
    wv_ext = nc.dram_tensor("wv", [C, C], F16, kind="ExternalInput")
    wvT_ext = nc.dram_tensor("wvT", [C, C], F16, kind="ExternalInput")
    pwT_ext = nc.dram_tensor("pwT", [C, C], F16, kind="ExternalInput")
    adjc_ext = nc.dram_tensor("adjc", [P, P], F32, kind="ExternalInput")
    out_ext = nc.dram_tensor("out", [C, NS], F32, kind="ExternalOutput")

    with tile.TileContext(nc) as tc:
        _body(tc, x_ext, gn_w_ext, gn_b_ext, qkv_b_ext, proj_b_ext,
              wqT_ext, wkT_ext, wv_ext, wvT_ext, pwT_ext, adjc_ext, out_ext)

    nc.compile()
    return nc


def _ap(t):
    return t if isinstance(t, bass.AP) else t[:]


def _body(tc, x_ext, gn_w_ext, gn_b_ext, qkv_b_ext, proj_b_ext,
          wqT_ext, wkT_ext, wv_ext, wvT_ext, pwT_ext, adjc_ext, out_ext):
    nc = tc.nc
    AX = mybir.AxisListType
    OP = mybir.AluOpType
    ACTF = mybir.ActivationFunctionType

    x_view = _ap(x_ext).rearrange("(ct p) n -> p ct n", p=P)      # [128,4,NS]
    out_view = _ap(out_ext).rearrange("(ct p) n -> p ct n", p=P)

    ctx = ExitStack()
    consts = ctx.enter_context(tc.tile_pool(name="consts", bufs=1))
    small = ctx.enter_context(tc.tile_pool(name="small", bufs=1))
    wpool = ctx.enter_context(tc.tile_pool(name="wpool", bufs=1))
    xbf_pool = ctx.enter_context(tc.tile_pool(name="xbf", bufs=1))
    p8a = ctx.enter_context(tc.tile_pool(name="p8a", bufs=1))
    p8b = ctx.enter_context(tc.tile_pool(name="p8b", bufs=1))
    stg = ctx.enter_context(tc.tile_pool(name="stg", bufs=3))
    chain = ctx.enter_context(tc.tile_pool(name="chain", bufs=4))
    ya_pool = ctx.enter_context(tc.tile_pool(name="ya", bufs=4))
    yo_pool = ctx.enter_context(tc.tile_pool(name="yo", bufs=4))
    ps_g = ctx.enter_context(tc.tile_pool(name="psg", bufs=4, space="PSUM"))
    ps_x = ctx.enter_context(tc.tile_pool(name="psx", bufs=4, space="PSUM"))
    dram = ctx.enter_context(tc.tile_pool(name="dram", bufs=1, space="DRAM"))

    # ---------------- constants / weights ----------------
    ident = consts.tile([P, P], F32, name="ident")
    make_identity(nc, ident)
    ident_h = consts.tile([P, P], F16, name="ident_h")
    nc.vector.tensor_copy(ident_h, ident)

    adj = consts.tile([P, P], F32, name="adj")          # 16x16 block-diag ones
    nc.sync.dma_start(adj, _ap(adjc_ext))

    gw_sb = consts.tile([P, CT], F32, name="gw_sb")
    gb_sb = consts.tile([P, CT], F32, name="gb_sb")
    pb_sb = consts.tile([P, CT], F32, name="pb_sb")
    nc.sync.dma_start(gw_sb, _ap(gn_w_ext).rearrange("(t p) -> p t", p=P))
    nc.sync.dma_start(gb_sb, _ap(gn_b_ext).rearrange("(t p) -> p t", p=P))
    nc.sync.dma_start(pb_sb, _ap(proj_b_ext).rearrange("(t p) -> p t", p=P))
    qkvb_sb = consts.tile([P, 3 * CT], F32, name="qkvb_sb")
    nc.sync.dma_start(qkvb_sb, _ap(qkv_b_ext).rearrange("(t p) -> p t", p=P))

    # fp16 weights, host-pretransposed; on the scalar hwdge queue
    wqT = wpool.tile([P, CT, C], F16, name="wqT")
    wkT = wpool.tile([P, CT, C], F16, name="wkT")
    wv_sb = wpool.tile([P, CT, C], F16, name="wv_sb")
    wvT = wpool.tile([P, CT, C], F16, name="wvT")
    pwT = wpool.tile([P, CT, C], F16, name="pwT")
    for dst, src in ((wqT, wqT_ext), (wkT, wkT_ext), (wv_sb, wv_ext),
                     (wvT, wvT_ext), (pwT, pwT_ext)):
        nc.sync.dma_start(dst, _ap(src).rearrange("(t p) c -> p t c", p=P))

    # ---------------- phase 1 ----------------
    # x_bf: bf16 copy of x (cast rides the gpsimd DMA); xt8 hi/lo: fp8
    # transposed copies at scale 16, layout [n_lo, subchunk, c].
    x_bf = xbf_pool.tile([P, CT, NS], BF16, name="x_bf")
    xt_hi = p8a.tile([P, NS // P, C], FP8, name="xt_hi", tag="a")
    xt_lo = p8b.tile([P, NS // P, C], FP8, name="xt_lo", tag="b")
    s_acc = small.tile([P, PH1_ITERS, CT], F32, name="s_acc")
    s_dump = small.tile([P, PH1_CHUNK], BF16, name="s_dump")

    G_ps = [ps_g.tile([P, C], F32, name=f"G_ps{j}", tag="g") for j in range(CT)]
    X_ps = [ps_x.tile([P, C], F32, name=f"X_ps{j}", tag="x") for j in range(CT)]

    # rank-1 operand pads zeroed early (gpsimd queue is in-order; done here
    # so the zeroing doesn't queue behind the collectives)
    Lpad = consts.tile([P, CT, P], F16, name="Lpad")
    Rpad = consts.tile([P, C], F16, name="Rpad")
    nc.gpsimd.memset(Lpad, 0.0)
    nc.gpsimd.memset(Rpad, 0.0)

    for u in range(PH1_ITERS):
        sl = slice(u * PH1_CHUNK, (u + 1) * PH1_CHUNK)
        nc.gpsimd.dma_start(x_bf[:, :, sl], x_view[:, :, sl])  # f32->bf16
        for ct in range(CT):
            xts = stg.tile([P, NSUB, P], BF16, name=f"xts{u}_{ct}", tag="s")
            nc.sync.dma_start(xts, x_bf[:, ct, sl], transpose=True)
            hi_sl = xt_hi[:, u * NSUB:(u + 1) * NSUB, ct * P:(ct + 1) * P]
            lo_sl = xt_lo[:, u * NSUB:(u + 1) * NSUB, ct * P:(ct + 1) * P]
            nc.vector.tensor_scalar_mul(hi_sl, xts, SC)
            nc.vector.scalar_tensor_tensor(
                out=lo_sl, in0=xts, scalar=SC, in1=hi_sl,
                op0=OP.mult, op1=OP.subtract,
            )
            # channel sums on the scalar engine (vector is busy casting)
            nc.scalar.activation(
                s_dump, x_bf[:, ct, sl], ACTF.Identity,
                bias=0.0, scale=1.0, accum_out=s_acc[:, u, ct:ct + 1],
            )
        # G matmuls: DoubleRow fp8, two passes sharing the moving operand
        for q in range(NSUB // 2):
            uq = u * NSUB + 2 * q
            rhs_hi = xt_hi[:, uq:uq + 2, :]
            rhs_lo = xt_lo[:, uq:uq + 2, :]
            first = (u == 0 and q == 0)
            last = (u == PH1_ITERS - 1 and q == NSUB // 2 - 1)
            for j in range(CT):
                lhs = xt_hi[:, uq:uq + 2, j * P:(j + 1) * P]
                nc.tensor.matmul(G_ps[j], lhs, rhs_hi, start=first,
                                 stop=last, perf_mode=DR)
                nc.tensor.matmul(X_ps[j], lhs, rhs_lo, start=first,
                                 stop=last, perf_mode=DR)

    # ---------------- phase-1 tail: s, diag, G assembly ----------------
    s_sb = small.tile([P, CT], F32, name="s_sb")
    nc.vector.reduce_sum(
        s_sb, s_acc[:].rearrange("p u t -> p t u"), axis=AX.X
    )

    # X -> fp16 SBUF (descaled), X^T via PE transposes, assemble G16
    X_sb = chain.tile([P, CT, C], F16, name="X_sb", tag="c")
    for j in range(CT):
        nc.scalar.activation(X_sb[:, j, :], X_ps[j], ACTF.Identity,
                             bias=0.0, scale=SCG)
    G16 = chain.tile([P, CT, C], F16, name="G16", tag="c")
    gtmp = small.tile([P, C], F32, name="gtmp")
    for j in range(CT):
        xt_ps = ps_x.tile([P, C], F16, name=f"xtp{j}", tag="x")
        for k in range(CT):
            nc.tensor.transpose(xt_ps[:, k * P:(k + 1) * P],
                                X_sb[:, k, j * P:(j + 1) * P], ident_h)
        nc.vector.scalar_tensor_tensor(
            out=gtmp, in0=G_ps[j], scalar=SCG, in1=X_sb[:, j, :],
            op0=OP.mult, op1=OP.add,
        )
        nc.vector.tensor_tensor(out=G16[:, j, :], in0=gtmp, in1=xt_ps,
                                op=OP.add)

    CC = C * C
    cc_g_in = dram.tile([CC], F16, name="cc_g_in")
    cc_g_out = dram.tile([CC], F16, name="cc_g_out")
    nc.sync.dma_start(
        cc_g_in[:].rearrange("(j p d) -> p j d", p=P, d=C), G16
    )

    # local diag(G) via flat-AP read of cc_g_in; pack [s | diag], small AR
    diag16 = small.tile([P, CT], F16, name="diag16")
    diag_src = bass.AP(
        tensor=cc_g_in.tensor,
        offset=cc_g_in.offset,
        ap=[[C + 1, P], [C * P + P, CT]],
    )
    nc.sync.dma_start(diag16, diag_src)
    sd_in = small.tile([P, 2 * CT], F32, name="sd_in")
    nc.vector.tensor_scalar_mul(sd_in[:, 0:CT], s_sb, 1.0 / SC)
    nc.vector.tensor_copy(sd_in[:, CT:2 * CT], diag16)

    cc_sd_in = dram.tile([2 * C], F32, name="cc_sd_in")
    cc_sd_out = dram.tile([2 * C], F32, name="cc_sd_out")
    nc.sync.dma_start(cc_sd_in[:].rearrange("(p t) -> p t", t=2 * CT), sd_in)
    nc.gpsimd.collective_compute(
        "AllReduce", OP.add, ins=[cc_sd_in[:]], outs=[cc_sd_out[:]],
        replica_groups=REPLICA_GROUPS,
    )
    nc.gpsimd.collective_compute(
        "AllReduce", OP.add, ins=[cc_g_in[:]], outs=[cc_g_out[:]],
        replica_groups=REPLICA_GROUPS,
    )

    # ---------------- during-AR: x hi/lo casts + stats + rank-1 ----------
    x_hi = p8a.tile([P, CT, NS], FP8, name="x_hi", tag="a")
    x_lo = p8b.tile([P, CT, NS], FP8, name="x_lo", tag="b")
    for ct in range(CT):
        nc.vector.tensor_scalar_mul(x_hi[:, ct, :], x_bf[:, ct, :], SC)

    sd_sb = small.tile([P, 2 * CT], F32, name="sd_sb")
    nc.sync.dma_start(sd_sb, cc_sd_out[:].rearrange("(p t) -> p t", t=2 * CT))
    sbar = sd_sb[:, 0:CT]
    diag = sd_sb[:, CT:2 * CT]

    sd_stack = small.tile([P, CT, 2], F32, name="sd_stack")
    nc.vector.tensor_copy(sd_stack[:, :, 0], sbar)
    nc.vector.tensor_copy(sd_stack[:, :, 1], diag)
    gsd = small.tile([P, CT, 2], F32, name="gsd")
    for ct in range(CT):
        gsd_ps = ps_x.tile([P, 2], F32, name=f"gsd_ps{ct}", tag="x")
        nc.tensor.matmul(gsd_ps, adj, sd_stack[:, ct, :], start=True, stop=True)
        nc.vector.tensor_copy(gsd[:, ct, :], gsd_ps)

    invN = 1.0 / float(GSIZE * N)
    meanex2 = small.tile([P, CT, 2], F32, name="meanex2")
    nc.vector.tensor_scalar_mul(meanex2, gsd, invN)
    mean = meanex2[:, :, 0]
    ex2 = meanex2[:, :, 1]
    msq = small.tile([P, CT], F32, name="msq")
    nc.vector.tensor_mul(out=msq, in0=mean, in1=mean)
    var = small.tile([P, CT], F32, name="var")
    nc.vector.scalar_tensor_tensor(
        out=var, in0=ex2, scalar=EPS, in1=msq, op0=OP.add, op1=OP.subtract
    )
    sd_ = small.tile([P, CT], F32, name="sd_")
    nc.scalar.sqrt(sd_, var)
    rstd = small.tile([P, CT], F32, name="rstd")
    nc.vector.reciprocal(rstd, sd_)
    a_sb = small.tile([P, CT], F32, name="a_sb")
    nc.vector.tensor_mul(out=a_sb, in0=rstd, in1=gw_sb)
    aSA = small.tile([P, CT], F32, name="aSA")
    nc.vector.tensor_scalar_mul(aSA, a_sb, SA)
    ma = small.tile([P, CT], F32, name="ma")
    nc.vector.tensor_mul(out=ma, in0=mean, in1=a_sb)
    bvec = small.tile([P, CT], F32, name="bvec")
    nc.vector.tensor_tensor(out=bvec, in0=gb_sb, in1=ma, op=OP.subtract)
    bvec_h = small.tile([P, CT], F16, name="bvec_h")
    nc.vector.tensor_copy(bvec_h, bvec)
    u1 = small.tile([P, CT], F32, name="u1")
    nc.vector.tensor_mul(out=u1, in0=a_sb, in1=sbar)

    uv2 = small.tile([P, CT, 2], F16, name="uv2")
    nc.vector.tensor_copy(uv2[:, :, 0], u1)
    nc.vector.tensor_copy(uv2[:, :, 1], bvec)

    # tq/bq, tk/bk with UNscaled wqT/wkT (fp16)
    tb_q = small.tile([P, CT, 2], F32, name="tb_q")
    tb_k = small.tile([P, CT, 2], F32, name="tb_k")
    for tb, WT, bias_off in ((tb_q, wqT, 0), (tb_k, wkT, CT)):
        for j in range(CT):
            tb_ps = ps_x.tile([P, 2], F32, name=f"tb_ps{bias_off}_{j}", tag="x")
            for ct in range(CT):
                nc.tensor.matmul(
                    tb_ps, WT[:, ct, j * P:(j + 1) * P], uv2[:, ct, :],
                    start=(ct == 0), stop=(ct == CT - 1),
                )
            nc.vector.tensor_copy(tb[:, j, :], tb_ps)
            nc.vector.tensor_add(
                out=tb[:, j, 1:2], in0=tb[:, j, 1:2],
                in1=qkvb_sb[:, bias_off + j:bias_off + j + 1],
            )

    # scale wqT/wkT in place by a (per input-channel partition)
    for WT in (wqT, wkT):
        for ct in range(CT):
            nc.vector.tensor_scalar_mul(
                WT[:, ct, :], WT[:, ct, :], a_sb[:, ct:ct + 1]
            )

    # w3vb = Wv bvec + bv (fp16)
    w3v = small.tile([P, CT], F32, name="w3v")
    for j in range(CT):
        w3v_ps = ps_x.tile([P, 1], F32, name=f"w3v_ps{j}", tag="x")
        for ct in range(CT):
            nc.tensor.matmul(
                w3v_ps, wvT[:, ct, j * P:(j + 1) * P], bvec_h[:, ct:ct + 1],
                start=(ct == 0), stop=(ct == CT - 1),
            )
        nc.vector.tensor_copy(w3v[:, j:j + 1], w3v_ps)
    w3vb = small.tile([P, CT], F16, name="w3vb")
    nc.vector.tensor_tensor(out=w3vb, in0=w3v,
                            in1=qkvb_sb[:, 2 * CT:3 * CT], op=OP.add)

    # wk2 = tk + N*bk ; rank-1 padded operands (fp16)
    wk2 = small.tile([P, CT], F32, name="wk2")
    nc.vector.tensor_scalar(wk2, tb_k[:, :, 1], float(N), None, OP.mult)
    nc.vector.tensor_add(out=wk2, in0=wk2, in1=tb_k[:, :, 0])

    tbq_h = small.tile([P, CT, 2], F16, name="tbq_h")
    nc.vector.tensor_copy(tbq_h, tb_q)
    rstack = small.tile([P, CT, 2], F16, name="rstack")
    nc.vector.tensor_copy(rstack[:, :, 0], tb_k[:, :, 1])
    nc.vector.tensor_copy(rstack[:, :, 1], wk2)
    for j in range(CT):
        lt_ps = ps_x.tile([2, P], F16, name=f"lt_ps{j}", tag="x")
        nc.tensor.transpose(lt_ps, tbq_h[:, j, :], ident_h)
        nc.vector.tensor_copy(Lpad[0:2, j, :], lt_ps)
        rt_ps = ps_x.tile([2, P], F16, name=f"rt_ps{j}", tag="x")
        nc.tensor.transpose(rt_ps, rstack[:, j, :], ident_h)
        nc.vector.tensor_copy(Rpad[0:2, j * P:(j + 1) * P], rt_ps)

    # x_lo cast (vector; runs under the G AllReduce)
    for ct in range(CT):
        nc.vector.scalar_tensor_tensor(
            out=x_lo[:, ct, :], in0=x_bf[:, ct, :], scalar=SC,
            in1=x_hi[:, ct, :], op0=OP.mult, op1=OP.subtract,
        )

    # ---------------- chain (fp16) ----------------
    Gb_sb = chain.tile([P, CT, C], F16, name="Gb_sb", tag="c")
    nc.sync.dma_start(
        Gb_sb, cc_g_out[:].rearrange("(j p d) -> p j d", p=P, d=C)
    )

    # V = Gbar @ Wk'^T   [c part, o free]
    V_sb = chain.tile([P, CT, C], F16, name="V_sb", tag="c")
    for j in range(CT):
        V_ps = ps_g.tile([P, C], F32, name=f"V_ps{j}", tag="g")
        for dt in range(CT):
            nc.tensor.matmul(
                V_ps, Gb_sb[:, dt, j * P:(j + 1) * P], wkT[:, dt, :],
                start=(dt == 0), stop=(dt == CT - 1),
            )
        nc.scalar.copy(V_sb[:, j, :], V_ps)

    # S = Wq'^T... logits; softmax -> attn (fp16)
    attn = chain.tile([P, CT, C], F16, name="attn", tag="c")
    S_ps = [ps_g.tile([P, C], F32, name=f"S_ps{j}", tag="g") for j in range(CT)]
    for j in range(CT):
        for ct in range(CT):
            nc.tensor.matmul(
                S_ps[j], wqT[:, ct, j * P:(j + 1) * P], V_sb[:, ct, :],
                start=(ct == 0), stop=False,
            )
        nc.tensor.matmul(S_ps[j], Lpad[:, j, :], Rpad, start=False, stop=True)
        mx = small.tile([P, 1], F32, name=f"mx{j}")
        nc.vector.reduce_max(mx, S_ps[j], axis=AX.X)
        mb = small.tile([P, 1], F32, name=f"mb{j}")
        nc.vector.tensor_scalar_mul(mb, mx, -SCALE)
        rs = small.tile([P, 1], F32, name=f"rs{j}")
        nc.scalar.activation(
            attn[:, j, :], S_ps[j], ACTF.Exp,
            bias=mb, scale=SCALE, accum_out=rs,
        )
        rrec = small.tile([P, 1], F32, name=f"rrec{j}")
        nc.vector.reciprocal(rrec, rs)
        nc.vector.tensor_scalar_mul(attn[:, j, :], attn[:, j, :], rrec)

    # attnT
    attnT = chain.tile([P, CT, C], F16, name="attnT", tag="c")
    for ct in range(CT):
        at_ps = ps_x.tile([P, C], F16, name=f"at_ps{ct}", tag="x")
        for j in range(CT):
            nc.tensor.transpose(
                at_ps[:, j * P:(j + 1) * P], attn[:, j, ct * P:(ct + 1) * P],
                ident_h,
            )
        nc.scalar.copy(attnT[:, ct, :], at_ps)

    # M0 = attn @ Wv'
    M0 = chain.tile([P, CT, C], F16, name="M0", tag="c")
    for j in range(CT):
        M0_ps = ps_g.tile([P, C], F32, name=f"M0_ps{j}", tag="g")
        for ot in range(CT):
            nc.tensor.matmul(
                M0_ps, attnT[:, ot, j * P:(j + 1) * P], wv_sb[:, ot, :],
                start=(ot == 0), stop=(ot == CT - 1),
            )
        nc.scalar.copy(M0[:, j, :], M0_ps)

    # u = attn @ w3vb (delta path)
    u_sb = small.tile([P, CT], F16, name="u_sb")
    for j in range(CT):
        u_ps = ps_x.tile([P, 1], F32, name=f"u_ps{j}", tag="x")
        for dt in range(CT):
            nc.tensor.matmul(
                u_ps, attnT[:, dt, j * P:(j + 1) * P], w3vb[:, dt:dt + 1],
                start=(dt == 0), stop=(dt == CT - 1),
            )
        nc.vector.tensor_copy(u_sb[:, j:j + 1], u_ps)

    # A0T = M0^T @ Pw^T directly; A8 = fp8(a*SA*A0T) hi/lo
    A_hi8 = consts.tile([P, CT, C], FP8, name="A_hi8")
    A_lo8 = consts.tile([P, CT, C], FP8, name="A_lo8")
    for cj in range(CT):
        a0t_ps = ps_g.tile([P, C], F32, name=f"a0t_ps{cj}", tag="g")
        for mt in range(CT):
            nc.tensor.matmul(
                a0t_ps, M0[:, mt, cj * P:(cj + 1) * P], pwT[:, mt, :],
                start=(mt == 0), stop=(mt == CT - 1),
            )
        nc.vector.tensor_scalar_mul(
            A_hi8[:, cj, :], a0t_ps, aSA[:, cj:cj + 1]
        )
        nc.vector.scalar_tensor_tensor(
            out=A_lo8[:, cj, :], in0=a0t_ps, scalar=aSA[:, cj:cj + 1],
            in1=A_hi8[:, cj, :], op0=OP.mult, op1=OP.subtract,
        )

    # delta = Pw u + pb
    delta = small.tile([P, CT], F32, name="delta")
    for ij in range(CT):
        d_ps = ps_x.tile([P, 1], F32, name=f"d_ps{ij}", tag="x")
        for mt in range(CT):
            nc.tensor.matmul(
                d_ps, pwT[:, mt, ij * P:(ij + 1) * P], u_sb[:, mt:mt + 1],
                start=(mt == 0), stop=(mt == CT - 1),
            )
        nc.vector.tensor_tensor(out=delta[:, ij:ij + 1], in0=d_ps,
                                in1=pb_sb[:, ij:ij + 1], op=OP.add)

    # ---------------- phase 2: 3-pass fp8 DoubleRow ----------------
    INV_OUT = 1.0 / (SA * SC)
    NG = 8                      # chunks per psum group
    for j in range(CT):
        for g in range(PH2_ITERS // NG):
            y_ps = []
            for uu in range(NG):
                pool = ps_g if uu < 4 else ps_x
                y_ps.append(pool.tile([P, PH2_CHUNK], F32,
                                      name=f"y{j}_{g}_{uu}",
                                      tag="g" if uu < 4 else "x"))
            stats_list = []
            for q in range(CT // 2):
                stats_list.append((A_hi8[:, 2 * q:2 * q + 2,
                                         j * P:(j + 1) * P], x_hi, q))
                stats_list.append((A_hi8[:, 2 * q:2 * q + 2,
                                         j * P:(j + 1) * P], x_lo, q))
            for q in range(CT // 2):
                stats_list.append((A_lo8[:, 2 * q:2 * q + 2,
                                         j * P:(j + 1) * P], x_hi, q))
            for si, (lhs, xsrc, q) in enumerate(stats_list):
                for uu in range(NG):
                    u2 = g * NG + uu
                    sl = slice(u2 * PH2_CHUNK, (u2 + 1) * PH2_CHUNK)
                    nc.tensor.matmul(
                        y_ps[uu], lhs, xsrc[:, 2 * q:2 * q + 2, sl],
                        start=(si == 0), stop=(si == len(stats_list) - 1),
                        perf_mode=DR,
                    )
            for uu in range(NG):
                u2 = g * NG + uu
                sl = slice(u2 * PH2_CHUNK, (u2 + 1) * PH2_CHUNK)
                y_act = ya_pool.tile([P, PH2_CHUNK], F32,
                                     name=f"ya{j}_{g}_{uu}", tag="y")
                nc.scalar.activation(
                    y_act, y_ps[uu], ACTF.Identity,
                    bias=delta[:, j:j + 1], scale=INV_OUT,
                )
                y_out = yo_pool.tile([P, PH2_CHUNK], F32,
                                     name=f"yo{j}_{g}_{uu}", tag="y")
                nc.vector.tensor_tensor(out=y_out, in0=y_act,
                                        in1=x_bf[:, j, sl], op=OP.add)
                nc.sync.dma_start(out_view[:, j, sl], y_out)

    ctx.close()


_CACHED_NC = None


def _get_nc():
    global _CACHED_NC
    if _CACHED_NC is None:
        _CACHED_NC = build_graph()
    return _CACHED_NC


def make_in_maps(inputs):
    xf = np.ascontiguousarray(
        np.asarray(inputs["x"], dtype=np.float32).reshape(B, C, N)
    )
    qkv_w = np.asarray(inputs["qkv_w"], dtype=np.float32)
    proj_w = np.asarray(inputs["proj_w"], dtype=np.float32)
    rep = {
        k: np.ascontiguousarray(np.asarray(inputs[k], dtype=np.float32))
        for k in ("gn_w", "gn_b", "qkv_b", "proj_b")
    }
    rep["wqT"] = np.ascontiguousarray(qkv_w[0:C].T.astype(np.float16))
    rep["wkT"] = np.ascontiguousarray(qkv_w[C:2 * C].T.astype(np.float16))
    rep["wv"] = np.ascontiguousarray(qkv_w[2 * C:3 * C].astype(np.float16))
    rep["wvT"] = np.ascontiguousarray(qkv_w[2 * C:3 * C].T.astype(np.float16))
    rep["pwT"] = np.ascontiguousarray(proj_w.T.astype(np.float16))
    ii = np.arange(P) // GSIZE
    rep["adjc"] = np.ascontiguousarray(
        (ii[:, None] == ii[None, :]).astype(np.float32)
    )
    in_maps = []
    for i in range(NCORES):
        b, sh = divmod(i, SHARDS)
        m = {"x": np.ascontiguousarray(xf[b, :, sh * NS:(sh + 1) * NS])}
        m.update(rep)
        in_maps.append(m)
    return in_maps


def assemble(results, inputs):
    x = np.asarray(inputs["x"])
    out = np.empty((B, C, N), dtype=np.float32)
    for i in range(NCORES):
        b, sh = divmod(i, SHARDS)
        out[b, :, sh * NS:(sh + 1) * NS] = results[i]["out"]
    return out.reshape(x.shape)


def kernel(**inputs) -> np.ndarray:
    nc = _get_nc()
    res = run_bass_kernel_spmd(nc, make_in_maps(inputs), list(range(NCORES)))
    return assemble(res.results, inputs)


if __name__ == "__main__":
    # quick smoke: build only
    build_graph()
    print("build OK")


# revision 10
# speedup vs baseline: 1.2180x; 1.2180x over previous
"""Distributed Trainium2 kernel for nn_AttentionBlock (channel attention).

Algorithm (exact algebra, no approximation):
  The attention matrix is [C,C] with the contraction over N=H*W*D tokens.
  GroupNorm is a per-channel affine xn = a*x + b whose stats derive from
  per-channel sums s = x@1 and the Gram matrix G = x@x.T (diag(G) = sumsq).
  Everything downstream of G is [C,C]-sized:
      S    = Wq' G Wk'^T + rank-1 terms        (Wq' = Wq diag(a))
      attn = softmax(S/sqrt(C))
      out  = x + P attn Wv' x + delta 1^T
  So the kernel does: pass 1 (G + s, reduced over local N-shard), a ~1MB
  AllReduce over the 4 cores sharing a batch, a small on-chip [512,512]
  chain, and pass 2 (one [C,C]x[C,N] matmul + residual).

Matmuls use float32r (reduced-precision fp32 at full PE rate; measured
~1.5e-4 per-matmul error). The BIR verifier requires every f32r-matmul
input to be produced by an instruction whose output dtype is float32r,
so tiles on f32r paths are f32r-typed and loads are rounded in place.

Sharding: batch B=2 x sequence 4  ->  8 cores. replica groups [[0..3],[4..7]].
"""

from contextlib import ExitStack

import numpy as np

import concourse.bass as bass
import concourse.tile as tile
from concourse import bacc, mybir
from concourse.bass_utils import run_bass_kernel_spmd
from concourse.masks import make_identity
from concourse.bass import _add_dep_helper as add_dep

# Problem constants (hardcoded per harness contract)
B = 2
C = 512
N = 32768          # 32*32*32
NCORES = 8
SHARDS = 4         # sequence shards per batch
NS = N // SHARDS   # 8192 per-core tokens
GROUPS = 32
GSIZE = C // GROUPS  # 16
EPS = 1e-5
P = 128
CT = C // P        # 4 channel tiles
F32 = mybir.dt.float32
F32R = mybir.dt.float32r
BF16 = mybir.dt.bfloat16

PH1_CHUNK = 128
PH1_ITERS = NS // PH1_CHUNK     # 64
PH2_CHUNK = 512
PH2_ITERS = NS // PH2_CHUNK     # 16

REPLICA_GROUPS = [[0, 1, 2, 3], [4, 5, 6, 7]]
SCALE = 1.0 / float(np.sqrt(C))


def f32_(ap):
    return ap.bitcast(F32)


def r_(ap):
    return ap.bitcast(F32R)


def build_graph():
    nc = bacc.Bacc(
        "TRN2", target_bir_lowering=False, debug=False, num_devices=NCORES
    )

    x_ext = nc.dram_tensor("x", [C, NS], F32, kind="ExternalInput")
    gn_w_ext = nc.dram_tensor("gn_w", [C], F32, kind="ExternalInput")
    gn_b_ext = nc.dram_tensor("gn_b", [C], F32, kind="ExternalInput")
    qkv_w_ext = nc.dram_tensor("qkv_w", [3 * C, C], F32, kind="ExternalInput")
    qkv_b_ext = nc.dram_tensor("qkv_b", [3 * C], F32, kind="ExternalInput")
    proj_w_ext = nc.dram_tensor("proj_w", [C, C], F32, kind="ExternalInput")
    wqT_ext = nc.dram_tensor("wqT", [C, C], F32, kind="ExternalInput")
    wkT_ext = nc.dram_tensor("wkT", [C, C], F32, kind="ExternalInput")
    pwT_ext = nc.dram_tensor("pwT", [C, C], F32, kind="ExternalInput")
    proj_b_ext = nc.dram_tensor("proj_b", [C], F32, kind="ExternalInput")
    adjc_ext = nc.dram_tensor("adjc", [P, P], F32, kind="ExternalInput")
    out_ext = nc.dram_tensor("out", [C, NS], F32, kind="ExternalOutput")

    with tile.TileContext(nc) as tc:
        _body(tc, x_ext, gn_w_ext, gn_b_ext, qkv_w_ext, qkv_b_ext,
              proj_w_ext, proj_b_ext, adjc_ext, out_ext,
              wqT_ext, wkT_ext, pwT_ext)

    nc.compile()
    return nc


def _body(tc, x_ext, gn_w_ext, gn_b_ext, qkv_w_ext, qkv_b_ext,
          proj_w_ext, proj_b_ext, adjc_ext, out_ext,
          wqT_ext, wkT_ext, pwT_ext):
    nc = tc.nc
    AX = mybir.AxisListType
    OP = mybir.AluOpType
    ACTF = mybir.ActivationFunctionType

    x_view = x_ext[:].rearrange("(ct p) n -> p ct n", p=P)        # [128,4,NS]
    out_view = out_ext[:].rearrange("(ct p) n -> p ct n", p=P)

    ctx = ExitStack()
    consts = ctx.enter_context(tc.tile_pool(name="consts", bufs=1))
    small = ctx.enter_context(tc.tile_pool(name="small", bufs=1))
    wpool = ctx.enter_context(tc.tile_pool(name="wpool", bufs=1))
    xres_pool = ctx.enter_context(tc.tile_pool(name="xres", bufs=1))
    xt_pool = ctx.enter_context(tc.tile_pool(name="xt", bufs=3))
    chain = ctx.enter_context(tc.tile_pool(name="chain", bufs=3))
    gb_pool = ctx.enter_context(tc.tile_pool(name="gbp", bufs=2))
    y_pool = ctx.enter_context(tc.tile_pool(name="yp", bufs=3))
    ps_g = ctx.enter_context(tc.tile_pool(name="psg", bufs=4, space="PSUM"))
    ps_t = ctx.enter_context(tc.tile_pool(name="pst", bufs=2, space="PSUM"))
    ps_y = ctx.enter_context(tc.tile_pool(name="psy", bufs=2, space="PSUM"))
    dram = ctx.enter_context(tc.tile_pool(name="dram", bufs=1, space="DRAM"))

    # ---------------- constants ----------------
    ident = consts.tile([P, P], F32, name="ident")
    make_identity(nc, ident)
    ident_r = consts.tile([P, P], F32R, name="ident_r")
    nc.vector.tensor_copy(ident_r, ident)
    ident_bf = consts.tile([P, P], BF16, name="ident_bf")
    nc.vector.tensor_copy(ident_bf, ident)

    adj = consts.tile([P, P], F32, name="adj")          # 16x16 block-diag ones
    nc.sync.dma_start(adj, adjc_ext[:])

    gw_sb = consts.tile([P, CT], F32, name="gw_sb")
    gb_sb = consts.tile([P, CT], F32, name="gb_sb")
    pb_sb = consts.tile([P, CT], F32, name="pb_sb")
    nc.sync.dma_start(gw_sb, gn_w_ext[:].rearrange("(t p) -> p t", p=P))
    nc.sync.dma_start(gb_sb, gn_b_ext[:].rearrange("(t p) -> p t", p=P))
    nc.sync.dma_start(pb_sb, proj_b_ext[:].rearrange("(t p) -> p t", p=P))
    qkvb_sb = consts.tile([P, 3 * CT], F32, name="qkvb_sb")
    nc.sync.dma_start(qkvb_sb, qkv_b_ext[:].rearrange("(t p) -> p t", p=P))

    # ------- phase 1: G = x x^T (bf16), s = x @ 1 -------
    # x is stored bf16 in SBUF; the cast rides the (gpsimd) DMA for free.
    x_res = xres_pool.tile([P, CT, NS], BF16, name="x_res")
    s_acc = consts.tile([P, PH1_ITERS, CT], F32, name="s_acc")

    G_ps = [ps_g.tile([P, C], F32, name=f"G_ps{ct}", tag="g") for ct in range(CT)]

    g_mms = []
    for u in range(PH1_ITERS):
        sl = slice(u * PH1_CHUNK, (u + 1) * PH1_CHUNK)
        xs = x_res[:, :, sl]
        nc.gpsimd.dma_start(xs, x_view[:, :, sl])   # casting DMA f32->bf16
        xt_ps = ps_t.tile([P, C], BF16, name=f"xt_ps{u}", tag="pt")
        for ct in range(CT):
            nc.tensor.transpose(xt_ps[:, ct * P:(ct + 1) * P],
                                xs[:, ct, :], ident_bf)
        xt = xt_pool.tile([P, C], BF16, name=f"xt{u}", tag="xt")
        nc.scalar.copy(xt, xt_ps)
        for ct in range(CT):
            mm = nc.tensor.matmul(
                G_ps[ct],
                xt[:, ct * P:(ct + 1) * P],
                xt[:],
                start=(u == 0),
                stop=(u == PH1_ITERS - 1),
            )
            g_mms.append(mm)
        nc.vector.reduce_sum(s_acc[:, u, :], xs, axis=AX.X)

    s_sb = small.tile([P, CT], F32, name="s_sb")
    nc.vector.reduce_sum(
        s_sb, s_acc[:].rearrange("p u t -> p t u"), axis=AX.X
    )

    # ---------------- AllReduce of [G | s] ----------------
    CC = C * C
    cc_in = dram.tile([CC + C], F32, name="cc_in")
    cc_out = dram.tile([CC + C], F32, name="cc_out")

    G_sb = chain.tile([P, CT, C], F32, name="G_sb", tag="c8")
    for ct in range(CT):
        nc.scalar.copy(G_sb[:, ct, :], G_ps[ct])
    nc.sync.dma_start(
        cc_in[0:CC].rearrange("(ct p d) -> p ct d", p=P, d=C), G_sb
    )
    nc.sync.dma_start(
        cc_in[CC:CC + C].rearrange("(p t) -> p t", t=CT), s_sb
    )

    nc.gpsimd.collective_compute(
        "AllReduce",
        OP.add,
        ins=[cc_in[:]],
        outs=[cc_out[:]],
        replica_groups=REPLICA_GROUPS,
    )

    # ------- weights host-pretransposed; round to f32r on-chip -------
    # WqT / WkT: [c_in partition, ct, o free]; PwT: [m partition, mt, o free]
    WqT = wpool.tile([P, CT, C], F32R, name="WqT")
    WkT = wpool.tile([P, CT, C], F32R, name="WkT")
    PwT = wpool.tile([P, CT, C], F32R, name="PwT")
    for Wdst, src_ap, label in (
        (WqT, wqT_ext[:], "wq"),
        (WkT, wkT_ext[:], "wk"),
        (PwT, pwT_ext[:], "pw"),
    ):
        w_stage = chain.tile([P, CT, C], F32, name=f"stage_{label}", tag="c8")
        nc.sync.dma_start(w_stage, src_ap.rearrange("(j p) c -> p j c", p=P))
        for ct in range(CT):
            nc.vector.tensor_copy(Wdst[:, ct, :], w_stage[:, ct, :])

    # modest bf16 warm-keepers through the AR wait
    dummy_ps = ps_t.tile([P, C], F32, name="dummy_ps", tag="pt")
    N_DUMMY = 20
    for i in range(N_DUMMY):
        nc.tensor.matmul(
            dummy_ps,
            x_res[:, 0, 0:P],
            x_res[:, 1, 0:C],
            start=(i == 0),
            stop=(i == N_DUMMY - 1),
        )
    dummy_sb = small.tile([P, 1], F32, name="dummy_sb")
    nc.vector.tensor_copy(dummy_sb, dummy_ps[:, 0:1])
    dummy_dram = dram.tile([P], F32, name="dummy_dram")
    nc.sync.dma_start(dummy_dram[:].rearrange("(p o) -> p o", o=1), dummy_sb)

    sbar = small.tile([P, CT], F32, name="sbar")
    nc.sync.dma_start(sbar, cc_out[CC:CC + C].rearrange("(p t) -> p t", t=CT))
    diag = small.tile([P, CT], F32, name="diag")
    # diagonal of Gbar: element (p, t) at flat offset p*(C+1) + t*(C*P + P)
    diag_src = bass.AP(
        tensor=cc_out.tensor,
        offset=cc_out.offset,
        ap=[[C + 1, P], [C * P + P, CT]],
    )
    nc.sync.dma_start(diag, diag_src)

    # ---------------- stats -> a, bvec ----------------
    sd_stack = small.tile([P, CT, 2], F32, name="sd_stack")
    nc.vector.tensor_copy(sd_stack[:, :, 0], sbar)
    nc.vector.tensor_copy(sd_stack[:, :, 1], diag)

    gsd = small.tile([P, CT, 2], F32, name="gsd")
    for ct in range(CT):
        gsd_ps = ps_t.tile([P, 2], F32, name=f"gsd_ps{ct}", tag="pt")
        nc.tensor.matmul(gsd_ps, adj, sd_stack[:, ct, :], start=True, stop=True)
        nc.vector.tensor_copy(gsd[:, ct, :], gsd_ps)

    invN = 1.0 / float(GSIZE * N)
    meanex2 = small.tile([P, CT, 2], F32, name="meanex2")
    nc.vector.tensor_scalar_mul(meanex2, gsd, invN)
    mean = meanex2[:, :, 0]
    ex2 = meanex2[:, :, 1]
    msq = small.tile([P, CT], F32, name="msq")
    nc.vector.tensor_mul(out=msq, in0=mean, in1=mean)
    var = small.tile([P, CT], F32, name="var")
    # var + eps = (ex2 + eps) - mean^2
    nc.vector.scalar_tensor_tensor(
        out=var, in0=ex2, scalar=EPS, in1=msq, op0=OP.add, op1=OP.subtract
    )
    sd_ = small.tile([P, CT], F32, name="sd_")
    nc.scalar.sqrt(sd_, var)
    rstd = small.tile([P, CT], F32, name="rstd")
    nc.vector.reciprocal(rstd, sd_)
    a_sb = small.tile([P, CT], F32, name="a_sb")
    nc.vector.tensor_mul(out=a_sb, in0=rstd, in1=gw_sb)
    ma = small.tile([P, CT], F32, name="ma")
    nc.vector.tensor_mul(out=ma, in0=mean, in1=a_sb)
    bvec = small.tile([P, CT], F32, name="bvec")
    nc.vector.tensor_tensor(out=bvec, in0=gb_sb, in1=ma, op=OP.subtract)
    u1 = small.tile([P, CT], F32, name="u1")
    nc.vector.tensor_mul(out=u1, in0=a_sb, in1=sbar)

    uv2 = small.tile([P, CT, 2], F32, name="uv2")
    nc.vector.tensor_copy(uv2[:, :, 0], u1)
    nc.vector.tensor_copy(uv2[:, :, 1], bvec)

    # ---------------- tq/bq, tk/bk (use UNscaled WqT/WkT) ----------------
    # tb[:, j, 0] = W(a*s); tb[:, j, 1] = W bvec (+ qkv bias)
    tb_q = small.tile([P, CT, 2], F32, name="tb_q")
    tb_k = small.tile([P, CT, 2], F32, name="tb_k")
    for tb, WT, bias_off in ((tb_q, WqT, 0), (tb_k, WkT, CT)):
        for j in range(CT):
            tb_ps = ps_t.tile([P, 2], F32, name=f"tb_ps{bias_off}_{j}", tag="pt")
            for ct in range(CT):
                nc.tensor.matmul(
                    tb_ps,
                    f32_(WT[:, ct, j * P:(j + 1) * P]),
                    uv2[:, ct, :],
                    start=(ct == 0),
                    stop=(ct == CT - 1),
                )
            nc.vector.tensor_copy(tb[:, j, :], tb_ps)
            nc.vector.tensor_add(
                out=tb[:, j, 1:2],
                in0=tb[:, j, 1:2],
                in1=qkvb_sb[:, bias_off + j:bias_off + j + 1],
            )

    # scale WqT/WkT in place by a (per input-channel partition)
    for WT in (WqT, WkT):
        for ct in range(CT):
            nc.vector.tensor_scalar_mul(
                WT[:, ct, :], f32_(WT[:, ct, :]), a_sb[:, ct:ct + 1]
            )

    # wk2 = tk + N*bk
    wk2 = small.tile([P, CT], F32, name="wk2")
    nc.vector.tensor_scalar(
        wk2, tb_k[:, :, 1], float(N), None, OP.mult
    )
    nc.vector.tensor_add(out=wk2, in0=wk2, in1=tb_k[:, :, 0])

    # ---------------- rank-1 padded operands ----------------
    # Lpad[0,j,:]=tq(j-slice), Lpad[1,j,:]=bq ; Rpad[0,:]=bk^T, Rpad[1,:]=wk2^T
    Lpad = consts.tile([P, CT, P], F32, name="Lpad")
    Rpad = consts.tile([P, C], F32, name="Rpad")
    nc.gpsimd.memset(Lpad, 0.0)
    nc.gpsimd.memset(Rpad, 0.0)

    rstack = small.tile([P, CT, 2], F32, name="rstack")
    nc.vector.tensor_copy(rstack[:, :, 0], tb_k[:, :, 1])
    nc.vector.tensor_copy(rstack[:, :, 1], wk2)

    for j in range(CT):
        lt_ps = ps_t.tile([2, P], F32, name=f"lt_ps{j}", tag="pt")
        nc.tensor.transpose(lt_ps, tb_q[:, j, :], ident)
        nc.vector.tensor_copy(Lpad[0:2, j, :], lt_ps)
        rt_ps = ps_t.tile([2, P], F32, name=f"rt_ps{j}", tag="pt")
        nc.tensor.transpose(rt_ps, rstack[:, j, :], ident)
        nc.vector.tensor_copy(Rpad[0:2, j * P:(j + 1) * P], rt_ps)

    # ---------------- V = G @ WkT_a   [c part, ok free] ----------------
    V_ps = [ps_g.tile([P, C], F32, name=f"V_ps{j}", tag="g") for j in range(CT)]
    for dt in range(CT):
        gb_stage = gb_pool.tile([P, C], F32, name=f"gbs{dt}", tag="gb")
        nc.sync.dma_start(
            gb_stage,
            cc_out[0:CC].rearrange("(ct p d) -> p ct d", p=P, d=C)[:, dt, :],
        )
        gb_t = gb_pool.tile([P, C], F32R, name=f"gb{dt}", tag="gb")
        nc.vector.tensor_copy(gb_t, gb_stage)     # rounding copy
        for j in range(CT):
            nc.tensor.matmul(
                V_ps[j],
                gb_t[:, j * P:(j + 1) * P],
                WkT[:, dt, :],
                start=(dt == 0),
                stop=(dt == CT - 1),
            )
    V_sb = chain.tile([P, CT, C], F32R, name="V_sb", tag="c8")
    for j in range(CT):
        nc.scalar.copy(V_sb[:, j, :], V_ps[j])

    # ---------------- S = WqT_a^T @ V + rank1 ; softmax ----------------
    attn = chain.tile([P, CT, C], F32R, name="attn", tag="c8")
    S_ps = [ps_g.tile([P, C], F32, name=f"S_ps{j}", tag="g") for j in range(CT)]
    for j in range(CT):
        for ct in range(CT):
            nc.tensor.matmul(
                S_ps[j],
                WqT[:, ct, j * P:(j + 1) * P],
                V_sb[:, ct, :],
                start=(ct == 0),
                stop=False,
            )
        nc.tensor.matmul(
            S_ps[j], Lpad[:, j, :], Rpad, start=False, stop=True
        )
        mx = small.tile([P, 1], F32, name=f"mx{j}")
        nc.vector.reduce_max(mx, S_ps[j], axis=AX.X)
        mb = small.tile([P, 1], F32, name=f"mb{j}")
        nc.vector.tensor_scalar_mul(mb, mx, -SCALE)
        rs = small.tile([P, 1], F32, name=f"rs{j}")
        nc.scalar.activation(
            attn[:, j, :], S_ps[j], ACTF.Exp,
            bias=mb, scale=SCALE, accum_out=rs,
        )
        rrec = small.tile([P, 1], F32, name=f"rrec{j}")
        nc.vector.reciprocal(rrec, rs)
        nc.vector.tensor_scalar_mul(attn[:, j, :], f32_(attn[:, j, :]), rrec)

    # ---------------- attnT ----------------
    attnT = chain.tile([P, CT, C], F32R, name="attnT", tag="c8")
    for ct in range(CT):
        at_ps = ps_t.tile([P, C], F32, name=f"at_ps{ct}", tag="pt")
        for j in range(CT):
            nc.tensor.transpose(
                r_(at_ps[:, j * P:(j + 1) * P]),
                attn[:, j, ct * P:(ct + 1) * P],
                ident_r,
            )
        nc.scalar.copy(attnT[:, ct, :], at_ps)

    # ---------------- M0 = attn @ Wv ----------------
    Wv_r = chain.tile([P, CT, C], F32R, name="Wv_r", tag="c8")
    for ot in range(CT):
        wv_stage = gb_pool.tile([P, C], F32, name=f"wvs{ot}", tag="gb")
        nc.sync.dma_start(
            wv_stage,
            qkv_w_ext[2 * C:3 * C, :].rearrange("(j p) c -> p j c", p=P)[:, ot, :],
        )
        nc.vector.tensor_copy(Wv_r[:, ot, :], wv_stage)  # cast -> f32r

    M0 = chain.tile([P, CT, C], F32R, name="M0", tag="c8")
    M0_ps = [ps_g.tile([P, C], F32, name=f"M0_ps{j}", tag="g") for j in range(CT)]
    for j in range(CT):
        for ot in range(CT):
            nc.tensor.matmul(
                M0_ps[j],
                attnT[:, ot, j * P:(j + 1) * P],
                Wv_r[:, ot, :],
                start=(ot == 0),
                stop=(ot == CT - 1),
            )
        nc.scalar.copy(M0[:, j, :], M0_ps[j])

    # w3 = attn @ bv
    w3 = small.tile([P, CT], F32, name="w3")
    for j in range(CT):
        w3_ps = ps_t.tile([P, 1], F32, name=f"w3_ps{j}", tag="pt")
        for ot in range(CT):
            nc.tensor.matmul(
                w3_ps,
                f32_(attnT[:, ot, j * P:(j + 1) * P]),
                qkvb_sb[:, 2 * CT + ot:2 * CT + ot + 1],
                start=(ot == 0),
                stop=(ot == CT - 1),
            )
        nc.vector.tensor_copy(w3[:, j:j + 1], w3_ps)


    # pw3 = Pw @ w3
    pw3 = small.tile([P, CT], F32, name="pw3")
    for j in range(CT):
        pw3_ps = ps_t.tile([P, 1], F32, name=f"pw3_ps{j}", tag="pt")
        for mt in range(CT):
            nc.tensor.matmul(
                pw3_ps,
                f32_(PwT[:, mt, j * P:(j + 1) * P]),
                w3[:, mt:mt + 1],
                start=(mt == 0),
                stop=(mt == CT - 1),
            )
        nc.vector.tensor_copy(pw3[:, j:j + 1], pw3_ps)

    # ------- A0T = M0^T @ PwT directly (no A0 round trip) -------
    A0T = chain.tile([P, CT, C], F32R, name="A0T", tag="c8")
    for cj in range(CT):
        a0t_ps = ps_g.tile([P, C], F32, name=f"a0t_ps{cj}", tag="g")
        for mt in range(CT):
            nc.tensor.matmul(
                a0t_ps,
                M0[:, mt, cj * P:(cj + 1) * P],
                PwT[:, mt, :],
                start=(mt == 0),
                stop=(mt == CT - 1),
            )
        nc.scalar.copy(A0T[:, cj, :], a0t_ps)

    ab = small.tile([P, CT], F32, name="ab")
    for j in range(CT):
        ab_ps = ps_t.tile([P, 1], F32, name=f"ab_ps{j}", tag="pt")
        for ct in range(CT):
            nc.tensor.matmul(
                ab_ps,
                f32_(A0T[:, ct, j * P:(j + 1) * P]),
                bvec[:, ct:ct + 1],
                start=(ct == 0),
                stop=(ct == CT - 1),
            )
        nc.vector.tensor_copy(ab[:, j:j + 1], ab_ps)

    delta = small.tile([P, CT], F32, name="delta")
    nc.vector.tensor_add(out=delta, in0=ab, in1=pw3)
    nc.vector.tensor_add(out=delta, in0=delta, in1=pb_sb)

    # lhsT_A = I + diag(a) @ A0T (row scale; residual identity folded in),
    # cast to bf16 for phase 2
    A_bf = consts.tile([P, CT, C], BF16, name="A_bf")
    A_f = consts.tile([P, CT, C], F32, name="A_f")
    for ct in range(CT):
        nc.vector.tensor_scalar_mul(
            A_f[:, ct, :], f32_(A0T[:, ct, :]), a_sb[:, ct:ct + 1]
        )
        # add identity on the diagonal block: A_f[:, ct, ct*P:(ct+1)*P] += I
        nc.vector.tensor_add(
            out=A_f[:, ct, ct * P:(ct + 1) * P],
            in0=A_f[:, ct, ct * P:(ct + 1) * P],
            in1=ident,
        )
        nc.vector.tensor_copy(A_bf[:, ct, :], A_f[:, ct, :])

    # ------- phase 2: out = (I + A) x + delta  (bf16 matmul, ACT epilogue) --
    for u in range(PH2_ITERS):
        sl = slice(u * PH2_CHUNK, (u + 1) * PH2_CHUNK)
        for j in range(CT):
            y_ps = ps_y.tile([P, PH2_CHUNK], F32, name=f"y_ps{u}_{j}", tag="y")
            for ct in range(CT):
                nc.tensor.matmul(
                    y_ps,
                    A_bf[:, ct, j * P:(j + 1) * P],
                    x_res[:, ct, sl],
                    start=(ct == 0),
                    stop=(ct == CT - 1),
                )
            y_sb = y_pool.tile([P, PH2_CHUNK], F32, name=f"y_sb{u}_{j}", tag="y")
            nc.scalar.activation(
                y_sb, y_ps, ACTF.Identity,
                bias=delta[:, j:j + 1], scale=1.0,
            )
            nc.sync.dma_start(out_view[:, j, sl], y_sb)

    ctx.close()


_CACHED_NC = None


def _get_nc():
    global _CACHED_NC
    if _CACHED_NC is None:
        _CACHED_NC = build_graph()
    return _CACHED_NC


def make_in_maps(inputs):
    xf = np.ascontiguousarray(
        np.asarray(inputs["x"], dtype=np.float32).reshape(B, C, N)
    )
    rep = {
        k: np.ascontiguousarray(np.asarray(inputs[k], dtype=np.float32))
        for k in ("gn_w", "gn_b", "qkv_w", "qkv_b", "proj_w", "proj_b")
    }
    qw = np.asarray(inputs["qkv_w"], dtype=np.float32)
    rep["wqT"] = np.ascontiguousarray(qw[0:C].T)
    rep["wkT"] = np.ascontiguousarray(qw[C:2 * C].T)
    rep["pwT"] = np.ascontiguousarray(
        np.asarray(inputs["proj_w"], dtype=np.float32).T)
    ii = np.arange(P) // GSIZE
    rep["adjc"] = np.ascontiguousarray(
        (ii[:, None] == ii[None, :]).astype(np.float32)
    )
    in_maps = []
    for i in range(NCORES):
        b, sh = divmod(i, SHARDS)
        m = {"x": np.ascontiguousarray(xf[b, :, sh * NS:(sh + 1) * NS])}
        m.update(rep)
        in_maps.append(m)
    return in_maps


def assemble(results, inputs):
    x = np.asarray(inputs["x"])
    out = np.empty((B, C, N), dtype=np.float32)
    for i in range(NCORES):
        b, sh = divmod(i, SHARDS)
        out[b, :, sh * NS:(sh + 1) * NS] = results[i]["out"]
    return out.reshape(x.shape)


def kernel(**inputs) -> np.ndarray:
    nc = _get_nc()
    res = run_bass_kernel_spmd(nc, make_in_maps(inputs), list(range(NCORES)))
    return assemble(res.results, inputs)


if __name__ == "__main__":
    # quick smoke: build only
    build_graph()
    print("build OK")

